# revision 5
# baseline (speedup 1.0000x reference)
"""BitLinear (ternary-weight linear with int8 activation quantization) on 8 trn2 cores.

y = (clip(round(x/x_scale),-128,127) * x_scale) @ (clip(round(w/w_scale),-1,1) * w_scale).T
  x_scale = max(max|x|, eps)/127   (per-tensor)
  w_scale = max(mean|w|, eps)      (per-tensor)

Sharding: tensor-parallel over out_features (11008 = 8 x 1376), x replicated.
Single device launch per core: quantize (magic-number rounding) + exact-integer
bf16 matmul. Per-tensor scales are two scalar reductions; they are computed
host-side and passed in as constants, so the launch is pure streaming compute.

x is shipped to the device as f16: the int8 quantization grid step (absmax/127)
is ~44x coarser than f16 rounding at the grid scale, so f16 transport perturbs
round(x/x_scale) on only ~0.3% of elements by +-1 step (measured end-to-end
rel err ~2.5e-3 vs the 2e-2 budget) while halving x DMA bytes. w stays f32:
ternary rounding near the +-0.5 boundary is precision-critical.

The emission plan software-pipelines the startup: w slice 0 + x block 0 are
interleaved k-ordered so the PE starts within ~10us, early blocks run on the
slices already resident while the remaining w slices stream in, then catch-up
passes (which need no new x DMA) fill the PE while x prefetch rebuilds.
"""

import numpy as np
from contextlib import ExitStack

import concourse.bass as bass
import concourse.tile as tile
from concourse import bacc, mybir
from concourse.bass_utils import run_bass_kernel_spmd

# problem shapes (hardcoded per contract)
B, T, I, O = 4, 2048, 4096, 11008
TOK = B * T                  # 8192
N_CORES = 8
O_SH = O // N_CORES          # 1376
EPS = 1e-5
MAGIC = 12582912.0           # 1.5 * 2**23: fp32 add forces round-to-nearest-even int
F32 = mybir.dt.float32
F16 = mybir.dt.float16
BF16 = mybir.dt.bfloat16

# tiling
TB = 256                     # tokens per streaming block
NBLK = TOK // TB             # 32
KT = I // 128                # 32 k-tiles
CH = 4                       # k-tiles per x DMA chunk (CH*TB*2B*128 = 256KB)
NCH = KT // CH               # 8 chunks per block
WCH = 2                      # k-tiles per w chunk
NWCH = KT // WCH             # 16 chunks per slice
OB = (512, 512, 352)         # out-feature split per PSUM bank (sum = 1376)
OB_OFF = (0, 512, 1024)


def _interleave(a, b, na, nb):
    """merge two op lists, taking na from a then nb from b, repeating."""
    out, ia, ib = [], 0, 0
    while ia < len(a) or ib < len(b):
        out.extend(a[ia:ia + na]); ia += na
        out.extend(b[ib:ib + nb]); ib += nb
    return out


def _make_plan():
    """Emission plan: list of ('w', s, c) / ('x', tb, c) / ('mmb', tb, banks)."""
    ops = []
    W = lambda s: [("w", s, c) for c in range(NWCH)]
    X = lambda tb: [("x", tb, c) for c in range(NCH)]
    # startup: x block0 and w slice0 interleaved, k-ordered on both sides
    ops += [("x", 0, 0), ("x", 0, 1)]
    ops += _interleave(W(0), X(0)[2:] + X(1), 2, 2)
    ops += [("mmb", 0, (0,))]
    ops += [("mmb", 1, (0,))]
    # slice1 streams while blocks 2,3 load and run on slice0
    ops += _interleave(W(1), X(2) + X(3), 3, 1)
    ops += [("mmb", 2, (0,))]
    ops += [("mmb", 0, (1,)), ("mmb", 1, (1,))]      # catch-up: banked xq, no DMA
    ops += [("mmb", 3, (0,))]
    # slice2 streams while block 4 loads; catch-up work keeps PE busy
    ops += _interleave(W(2), X(4), 3, 1)
    ops += [("mmb", 2, (1,)), ("mmb", 3, (1,))]
    ops += [("mmb", 0, (2,)), ("mmb", 1, (2,))]
    ops += [("mmb", 2, (2,)), ("mmb", 3, (2,))]
    # steady state; X emitted after mmb so PSUM drains outrank quant work on
    # the scalar engine at equal readiness (runtime prefetch depth comes from
    # the xq pool slots, not emission order)
    for tb in range(4, NBLK):
        ops += [("mmb", tb, (0, 1, 2))]
        if tb + 1 < NBLK:
            ops += X(tb + 1)
    return ops


def _build_matmul(plan=None):
    nc = bacc.Bacc("TRN2", target_bir_lowering=False, debug=False,
                   num_devices=N_CORES)
    xT = nc.dram_tensor("xT", [I, TOK], F16, kind="ExternalInput").ap()
    wT = nc.dram_tensor("wT", [I, O_SH], F32, kind="ExternalInput").ap()
    consts = nc.dram_tensor("consts", [1, 8], F32, kind="ExternalInput").ap()
    out = nc.dram_tensor("out", [TOK, O_SH], F32, kind="ExternalOutput").ap()

    xTr = xT.rearrange("(kt p) t -> p kt t", p=128)   # [128, KT, TOK]
    wTr = wT.rearrange("(kt p) o -> p kt o", p=128)   # [128, KT, O_SH]

    if plan is None:
        plan = _make_plan()

    with tile.TileContext(nc) as tc:
        with ExitStack() as ctx:
            const_pool = ctx.enter_context(tc.tile_pool(name="const", bufs=1))
            wq_pool = ctx.enter_context(tc.tile_pool(name="wq", bufs=1))
            stage = ctx.enter_context(tc.tile_pool(name="stage", bufs=2))
            rnd = ctx.enter_context(tc.tile_pool(name="rnd", bufs=2))
            wstage = ctx.enter_context(tc.tile_pool(name="wstage", bufs=2))
            xq_pool = ctx.enter_context(tc.tile_pool(name="xq", bufs=5))
            out_pool = ctx.enter_context(tc.tile_pool(name="out", bufs=4))
            psum = ctx.enter_context(tc.tile_pool(name="psum", bufs=8, space="PSUM"))

            sb_c = const_pool.tile([128, 8], F32)
            nc.sync.dma_start(sb_c[:], consts.to_broadcast((128, 8)))
            inv_w = sb_c[:, 0:1]
            inv_x = sb_c[:, 1:2]
            out_scale = sb_c[:, 2:3]

            # SBUF-resident ternarized weight shard, bf16 [128, KT, O_SH]
            wq = wq_pool.tile([128, KT, O_SH], BF16)

            def quant_w_chunk(s, c):
                o0, ow = OB_OFF[s], OB[s]
                k0 = c * WCH
                wf = wstage.tile([128, WCH, ow], F32, tag="wstage",
                                 name=f"wf{s}_{c}")
                nc.sync.dma_start(wf[:], wTr[:, k0:k0 + WCH, o0:o0 + ow])
                # round(w * inv_w) in magic space (ACT: out = in*scale + bias)
                nc.scalar.activation(wf[:], wf[:],
                                     mybir.ActivationFunctionType.Copy,
                                     bias=MAGIC, scale=inv_w)
                # clip to [-1, 1] in magic space
                nc.vector.tensor_scalar(wf[:], wf[:], MAGIC + 1.0, MAGIC - 1.0,
                                        op0=mybir.AluOpType.min,
                                        op1=mybir.AluOpType.max)
                # subtract magic, cast bf16 into resident wq
                nc.vector.tensor_scalar(
                    wq[:, k0:k0 + WCH, o0:o0 + ow],
                    wf[:], -MAGIC, None, op0=mybir.AluOpType.add)

            xq_tiles = {}

            def quant_x_chunk(tb, c):
                t0 = tb * TB
                if tb not in xq_tiles:
                    xq_tiles[tb] = xq_pool.tile([128, KT, TB], BF16, tag="xq",
                                                name=f"xq{tb}")
                xq = xq_tiles[tb]
                k0 = c * CH
                xf = stage.tile([128, CH, TB], F16, tag="stage",
                                name=f"xf{tb}_{c}")
                nc.sync.dma_start(xf[:], xTr[:, k0:k0 + CH, t0:t0 + TB])
                xr = rnd.tile([128, CH, TB], F32, tag="rnd",
                              name=f"xr{tb}_{c}")
                nc.scalar.activation(xr[:], xf[:],
                                     mybir.ActivationFunctionType.Copy,
                                     bias=MAGIC, scale=inv_x)
                # no clip needed: |x|/x_scale <= 127 by construction
                nc.vector.tensor_scalar(
                    xq[:, k0:k0 + CH, :],
                    xr[:], -MAGIC, None, op0=mybir.AluOpType.add)

            def mm_block(tb, banks):
                """k-outer/bank-inner matmuls for both j-tiles of block tb."""
                xq = xq_tiles[tb]
                t0 = tb * TB
                for j in range(TB // 128):
                    ps = {}
                    for b in banks:
                        ps[b] = psum.tile([128, 512], F32, tag="ps",
                                          name=f"ps{tb}_{j}_{b}")
                    for k in range(KT):
                        for b in banks:
                            nc.tensor.matmul(ps[b][:, :OB[b]],
                                             xq[:, k, j * 128:(j + 1) * 128],
                                             wq[:, k, OB_OFF[b]:OB_OFF[b] + OB[b]],
                                             start=(k == 0), stop=(k == KT - 1))
                    for b in banks:
                        ob = out_pool.tile([128, 512], F32, tag="ob",
                                           name=f"ob{tb}_{j}_{b}")
                        nc.scalar.mul(ob[:, :OB[b]], ps[b][:, :OB[b]], out_scale)
                        nc.sync.dma_start(
                            out[t0 + j * 128:t0 + j * 128 + 128,
                                OB_OFF[b]:OB_OFF[b] + OB[b]],
                            ob[:, :OB[b]])

            for op in plan:
                if op[0] == "w":
                    quant_w_chunk(op[1], op[2])
                elif op[0] == "x":
                    quant_x_chunk(op[1], op[2])
                else:
                    mm_block(op[1], op[2])
    nc.compile()
    return nc


_cache = {}


def _get_nc():
    if "B" not in _cache:
        _cache["B"] = _build_matmul()
    return _cache["B"]


def _run(nc, in_maps, core_ids):
    try:
        return run_bass_kernel_spmd(nc, in_maps, core_ids)
    except Exception:
        import time as _t
        _t.sleep(10)  # transient tunnel/device hiccups recover on retry
        return run_bass_kernel_spmd(nc, in_maps, core_ids)


def kernel(x: np.ndarray, weight: np.ndarray) -> np.ndarray:
    ncB = _get_nc()
    core_ids = list(range(N_CORES))

    x = np.asarray(x)
    weight = np.asarray(weight)
    assert x.shape == (B, T, I) and weight.shape == (O, I), (x.shape, weight.shape)
    x_flat = x.reshape(TOK, I).astype(np.float32, copy=False)
    weight = np.ascontiguousarray(weight, dtype=np.float32)

    # per-tensor scales (two scalar reductions over the inputs)
    absmax = np.float32(np.abs(x_flat).max())
    wmean = np.float32(np.abs(weight).mean(dtype=np.float64))
    x_scale = np.float32(max(absmax, np.float32(EPS))) / np.float32(127.0)
    w_scale = np.float32(max(wmean, np.float32(EPS)))
    consts = np.zeros((1, 8), dtype=np.float32)
    consts[0, 0] = np.float32(1.0) / w_scale
    consts[0, 1] = np.float32(1.0) / x_scale
    consts[0, 2] = x_scale * w_scale

    # quantized matmul, tensor-parallel over out_features
    xT16 = np.asarray(x_flat.T, dtype=np.float16, order="C")   # [I, TOK]
    wTf = weight.T                                             # [I, O] view
    in_B = [{
        "xT": xT16,
        "wT": np.ascontiguousarray(wTf[:, i * O_SH:(i + 1) * O_SH]),
        "consts": consts,
    } for i in range(N_CORES)]
    resB = _run(ncB, in_B, core_ids)
    out = np.concatenate([resB.results[i]["out"] for i in range(N_CORES)], axis=1)
    return out.reshape(B, T, O)


# revision 10
# speedup vs baseline: 1.0280x; 1.0280x over previous
"""BitLinear (ternary-weight linear with int8 activation quantization) on 8 trn2 cores.

y = (clip(round(x/x_scale),-128,127) * x_scale) @ (clip(round(w/w_scale),-1,1) * w_scale).T
  x_scale = max(max|x|, eps)/127   (per-tensor)
  w_scale = max(mean|w|, eps)      (per-tensor)

Sharding: tensor-parallel over out_features (11008 = 8 x 1376), x replicated.
Single device launch per core: quantize (magic-number rounding) + exact-integer
bf16 matmul. Per-tensor scales are two scalar reductions; they are computed
host-side and passed in as constants, so the launch is pure streaming compute.

x is shipped to the device as f16: the int8 quantization grid step (absmax/127)
is ~44x coarser than f16 rounding at the grid scale, so f16 transport perturbs
round(x/x_scale) on only ~0.3% of elements by +-1 step (measured end-to-end
rel err ~2.5e-3 vs the 2e-2 budget) while halving x DMA bytes. w stays f32:
ternary rounding near the +-0.5 boundary is precision-critical.

The emission plan software-pipelines the startup: w slice 0 + x block 0 are
interleaved k-ordered so the PE starts within ~10us, early blocks run on the
slices already resident while the remaining w slices stream in, then catch-up
passes (which need no new x DMA) fill the PE while x prefetch rebuilds.
"""

import numpy as np
from contextlib import ExitStack

import concourse.bass as bass
import concourse.tile as tile
from concourse import bacc, mybir
from concourse.bass_utils import run_bass_kernel_spmd

# problem shapes (hardcoded per contract)
B, T, I, O = 4, 2048, 4096, 11008
TOK = B * T                  # 8192
N_CORES = 8
O_SH = O // N_CORES          # 1376
EPS = 1e-5
MAGIC = 12582912.0           # 1.5 * 2**23: fp32 add forces round-to-nearest-even int
F32 = mybir.dt.float32
F16 = mybir.dt.float16
BF16 = mybir.dt.bfloat16

# tiling
TB = 256                     # tokens per streaming block
NBLK = TOK // TB             # 32
KT = I // 128                # 32 k-tiles
CH = 4                       # k-tiles per x DMA chunk (CH*TB*2B*128 = 256KB)
NCH = KT // CH               # 8 chunks per block
WCH = 2                      # k-tiles per w chunk
NWCH = KT // WCH             # 16 chunks per slice
OB = (512, 512, 352)         # out-feature split per PSUM bank (sum = 1376)
OB_OFF = (0, 512, 1024)


def _interleave(a, b, na, nb):
    """merge two op lists, taking na from a then nb from b, repeating."""
    out, ia, ib = [], 0, 0
    while ia < len(a) or ib < len(b):
        out.extend(a[ia:ia + na]); ia += na
        out.extend(b[ib:ib + nb]); ib += nb
    return out


def _make_plan():
    """Emission plan: list of ('w', s, c) / ('x', tb, c) / ('mmb', tb, banks)."""
    ops = []
    W = lambda s: [("w", s, c) for c in range(NWCH)]
    X = lambda tb: [("x", tb, c) for c in range(NCH)]
    # startup: x block0 and w slice0 interleaved, k-ordered on both sides
    ops += [("x", 0, 0), ("x", 0, 1)]
    ops += _interleave(W(0), X(0)[2:] + X(1), 2, 2)
    ops += [("mmb", 0, (0,))]
    ops += [("mmb", 1, (0,))]
    # slice1 streams while blocks 2,3 load and run on slice0
    ops += _interleave(W(1), X(2) + X(3), 3, 1)
    ops += [("mmb", 2, (0,))]
    ops += [("mmb", 0, (1,)), ("mmb", 1, (1,))]      # catch-up: banked xq, no DMA
    ops += [("mmb", 3, (0,))]
    # slice2 streams while block 4 loads; catch-up work keeps PE busy
    ops += _interleave(W(2), X(4), 3, 1)
    ops += [("mmb", 2, (1,)), ("mmb", 3, (1,))]
    ops += [("mmb", 0, (2,)), ("mmb", 1, (2,))]
    ops += [("mmb", 2, (2,)), ("mmb", 3, (2,))]
    # steady state; X emitted after mmb so PSUM drains outrank quant work on
    # the scalar engine at equal readiness (runtime prefetch depth comes from
    # the xq pool slots, not emission order)
    for tb in range(4, NBLK):
        ops += [("mmb", tb, (0, 1, 2))]
        if tb + 1 < NBLK:
            ops += X(tb + 1)
    return ops


def _build_matmul(plan=None):
    nc = bacc.Bacc("TRN2", target_bir_lowering=False, debug=False,
                   num_devices=N_CORES)
    # x in block-major layout: [NBLK, 128, KT, TB] so every DMA chunk reads
    # 2KB-contiguous per-partition lines (f16 tokens of one block+k-tile)
    xb = nc.dram_tensor("xb", [NBLK * 128, KT * TB], F16,
                        kind="ExternalInput").ap()
    wT = nc.dram_tensor("wT", [I, O_SH], F32, kind="ExternalInput").ap()
    consts = nc.dram_tensor("consts", [1, 8], F32, kind="ExternalInput").ap()
    out = nc.dram_tensor("out", [TOK, O_SH], F32, kind="ExternalOutput").ap()

    wTr = wT.rearrange("(kt p) o -> p kt o", p=128)   # [128, KT, O_SH]

    if plan is None:
        plan = _make_plan()

    with tile.TileContext(nc) as tc:
        with ExitStack() as ctx:
            const_pool = ctx.enter_context(tc.tile_pool(name="const", bufs=1))
            wq_pool = ctx.enter_context(tc.tile_pool(name="wq", bufs=1))
            stage = ctx.enter_context(tc.tile_pool(name="stage", bufs=3))
            rnd = ctx.enter_context(tc.tile_pool(name="rnd", bufs=3))
            wstage = ctx.enter_context(tc.tile_pool(name="wstage", bufs=4))
            xq_pool = ctx.enter_context(tc.tile_pool(name="xq", bufs=5))
            out_pool = ctx.enter_context(tc.tile_pool(name="out", bufs=3))
            psum = ctx.enter_context(tc.tile_pool(name="psum", bufs=8, space="PSUM"))

            sb_c = const_pool.tile([128, 8], F32)
            nc.sync.dma_start(sb_c[:], consts.to_broadcast((128, 8)))
            inv_w = sb_c[:, 0:1]
            inv_x = sb_c[:, 1:2]
            out_scale = sb_c[:, 2:3]

            # SBUF-resident ternarized weight shard, bf16 [128, KT, O_SH]
            wq = wq_pool.tile([128, KT, O_SH], BF16)

            def quant_w_chunk(s, c):
                o0, ow = OB_OFF[s], OB[s]
                k0 = c * WCH
                wf = wstage.tile([128, WCH, ow], F32, tag="wstage",
                                 name=f"wf{s}_{c}")
                nc.sync.dma_start(wf[:], wTr[:, k0:k0 + WCH, o0:o0 + ow])
                # round(w * inv_w) in magic space (ACT: out = in*scale + bias)
                nc.scalar.activation(wf[:], wf[:],
                                     mybir.ActivationFunctionType.Copy,
                                     bias=MAGIC, scale=inv_w)
                # clip to [-1, 1] in magic space
                nc.vector.tensor_scalar(wf[:], wf[:], MAGIC + 1.0, MAGIC - 1.0,
                                        op0=mybir.AluOpType.min,
                                        op1=mybir.AluOpType.max)
                # subtract magic, cast bf16 into resident wq
                nc.vector.tensor_scalar(
                    wq[:, k0:k0 + WCH, o0:o0 + ow],
                    wf[:], -MAGIC, None, op0=mybir.AluOpType.add)

            xq_tiles = {}

            def quant_x_chunk(tb, c):
                if tb not in xq_tiles:
                    xq_tiles[tb] = xq_pool.tile([128, KT, TB], BF16, tag="xq",
                                                name=f"xq{tb}")
                xq = xq_tiles[tb]
                k0 = c * CH
                xf = stage.tile([128, CH, TB], F16, tag="stage",
                                name=f"xf{tb}_{c}")
                # x DMA issued from the Activation HWDGE queue set so the w
                # stream (sync) can't head-of-line block it
                nc.scalar.dma_start(
                    xf[:], xb[tb * 128:(tb + 1) * 128,
                              k0 * TB:(k0 + CH) * TB])
                xr = rnd.tile([128, CH, TB], F32, tag="rnd",
                              name=f"xr{tb}_{c}")
                nc.scalar.activation(xr[:], xf[:],
                                     mybir.ActivationFunctionType.Copy,
                                     bias=MAGIC, scale=inv_x)
                # no clip needed: |x|/x_scale <= 127 by construction
                nc.vector.tensor_scalar(
                    xq[:, k0:k0 + CH, :],
                    xr[:], -MAGIC, None, op0=mybir.AluOpType.add)

            def mm_block(tb, banks):
                """k-outer/bank-inner matmuls for both j-tiles of block tb."""
                xq = xq_tiles[tb]
                t0 = tb * TB
                for j in range(TB // 128):
                    ps = {}
                    for b in banks:
                        ps[b] = psum.tile([128, 512], F32, tag="ps",
                                          name=f"ps{tb}_{j}_{b}")
                    for k in range(KT):
                        for b in banks:
                            nc.tensor.matmul(ps[b][:, :OB[b]],
                                             xq[:, k, j * 128:(j + 1) * 128],
                                             wq[:, k, OB_OFF[b]:OB_OFF[b] + OB[b]],
                                             start=(k == 0), stop=(k == KT - 1))
                    for b in banks:
                        ob = out_pool.tile([128, 512], F32, tag="ob",
                                           name=f"ob{tb}_{j}_{b}")
                        nc.scalar.mul(ob[:, :OB[b]], ps[b][:, :OB[b]], out_scale)
                        nc.sync.dma_start(
                            out[t0 + j * 128:t0 + j * 128 + 128,
                                OB_OFF[b]:OB_OFF[b] + OB[b]],
                            ob[:, :OB[b]])

            for op in plan:
                if op[0] == "w":
                    quant_w_chunk(op[1], op[2])
                elif op[0] == "x":
                    quant_x_chunk(op[1], op[2])
                else:
                    mm_block(op[1], op[2])
    nc.compile()
    return nc


_cache = {}


def _get_nc():
    if "B" not in _cache:
        _cache["B"] = _build_matmul()
    return _cache["B"]


def _run(nc, in_maps, core_ids):
    try:
        return run_bass_kernel_spmd(nc, in_maps, core_ids)
    except Exception:
        import time as _t
        _t.sleep(10)  # transient tunnel/device hiccups recover on retry
        return run_bass_kernel_spmd(nc, in_maps, core_ids)


def kernel(x: np.ndarray, weight: np.ndarray) -> np.ndarray:
    ncB = _get_nc()
    core_ids = list(range(N_CORES))

    x = np.asarray(x)
    weight = np.asarray(weight)
    assert x.shape == (B, T, I) and weight.shape == (O, I), (x.shape, weight.shape)
    x_flat = x.reshape(TOK, I).astype(np.float32, copy=False)
    weight = np.ascontiguousarray(weight, dtype=np.float32)

    # per-tensor scales (two scalar reductions over the inputs)
    absmax = np.float32(np.abs(x_flat).max())
    wmean = np.float32(np.abs(weight).mean(dtype=np.float64))
    x_scale = np.float32(max(absmax, np.float32(EPS))) / np.float32(127.0)
    w_scale = np.float32(max(wmean, np.float32(EPS)))
    consts = np.zeros((1, 8), dtype=np.float32)
    consts[0, 0] = np.float32(1.0) / w_scale
    consts[0, 1] = np.float32(1.0) / x_scale
    consts[0, 2] = x_scale * w_scale

    # quantized matmul, tensor-parallel over out_features
    xT16 = np.asarray(x_flat.T, dtype=np.float16, order="C")   # [I, TOK]
    # block-major: [NBLK, 128, KT, TB] so device DMA lines are 2KB contiguous
    xb = np.ascontiguousarray(
        xT16.reshape(KT, 128, NBLK, TB).transpose(2, 1, 0, 3)
    ).reshape(NBLK * 128, KT * TB)
    wTf = weight.T                                             # [I, O] view
    in_B = [{
        "xb": xb,
        "wT": np.ascontiguousarray(wTf[:, i * O_SH:(i + 1) * O_SH]),
        "consts": consts,
    } for i in range(N_CORES)]
    resB = _run(ncB, in_B, core_ids)
    out = np.concatenate([resB.results[i]["out"] for i in range(N_CORES)], axis=1)
    return out.reshape(B, T, O)
